# revision 1
# baseline (speedup 1.0000x reference)
"""Trainium2 Bass kernel for nn_AttentionLayer_77524159693050 (retrieval_knn).

Math reduction used here (verified against the reference):
  e[n,k] = eX[n] + eC[n,k]; top_k with k=KC-1 then sort == drop argmin_k eC.
  eC[n,k] = Candidate[n,k,:]@v1 + data_m_train[neigh_ind[n,k],:]@v2
  where v = W @ A[OUT:,0], v1 = v[:DIM], v2 = v[DIM:].
  a_out/b_out only need s[j] = sum_d data_m_train[j,d] gathered at neigh_ind.

Device program 1 (SPMD x8): per-core shard of data_m_train -> (t,s) pair table.
Host: glue only - index lookup ts_table[neigh_ind] (fine-grained gather is not
supported by the hardware's indirect DMA, which is one-index-per-partition).
Device program 2 (SPMD x8): row-sharded over n: dot products, argmin, the
10-of-11 row selection, nd/ni selection, and a/b partial sums.
"""

import sys
import types
import numpy as np

sys.path.insert(0, "/opt/trn_rl_repo")

from concourse import bass, mybir, bacc  # noqa: E402
from concourse.tile import TileContext  # noqa: E402
from concourse.bass_utils import run_bass_kernel_spmd  # noqa: E402

f32 = mybir.dt.float32
i32 = mybir.dt.int32

NO, KC, DIM, NT, OUT, K = 20000, 11, 256, 100000, 128, 10
NCORES = 8
NO_SH = NO // NCORES          # 2500
NO_PAD = 2560                 # 20 tiles of 128
NT_SH = NT // NCORES          # 12500
NT_BLK = 98                   # blocks of 128 rows per core
NT_PAD = 128 * NT_BLK         # 12544
P = 128

TRACE = {"enabled": False, "exec_ns": []}

_cache = {}


def _ntff_hook():
    try:
        from trn_agent_boot import trn_boot
        hook = trn_boot._ntff_profile_via_ctypes("/opt/axon/libaxon_pjrt.so")
        mod = types.ModuleType("antenv.axon_hooks")
        mod.get_axon_ntff_profile_hook = lambda: hook
        sys.modules["antenv.axon_hooks"] = mod
        return True
    except Exception:
        return False


def build_p1():
    nc = bacc.Bacc()
    dmt_d = nc.declare_dram_parameter("dmt", [NT_PAD, DIM], f32, isOutput=False)
    v2r_d = nc.declare_dram_parameter("v2r", [P, DIM], f32, isOutput=False)
    tsloc_d = nc.declare_dram_parameter("tsloc", [NT_PAD, 2], f32, isOutput=True)

    GB = 7            # row-blocks of 128 per DMA group
    NG = NT_BLK // GB  # 14 groups

    dmt_v = dmt_d[:].rearrange("(i p) d -> p i d", p=P)       # [128, 98, 256]
    tsloc_v = tsloc_d[:].rearrange("(p i) c -> p i c", i=NT_BLK)  # [128, 98, 2]

    with TileContext(nc) as tc:
        with (
            tc.tile_pool(name="io", bufs=3) as io_pool,
            tc.tile_pool(name="work", bufs=2) as work_pool,
            tc.tile_pool(name="acc", bufs=1) as acc_pool,
        ):
            v2r_t = acc_pool.tile([P, DIM], f32)
            nc.sync.dma_start(out=v2r_t[:], in_=v2r_d[:])
            tsacc = acc_pool.tile([P, NT_BLK, 2], f32)
            for g in range(NG):
                in_t = io_pool.tile([P, GB, DIM], f32, tag="in")
                nc.sync.dma_start(out=in_t[:], in_=dmt_v[:, g * GB:(g + 1) * GB, :])
                prod_t = work_pool.tile([P, GB, DIM], f32, tag="prod")
                nc.vector.tensor_tensor(
                    out=prod_t[:], in0=in_t[:],
                    in1=v2r_t[:].unsqueeze(1).broadcast_to([P, GB, DIM]),
                    op=mybir.AluOpType.mult)
                nc.vector.tensor_reduce(
                    out=tsacc[:, g * GB:(g + 1) * GB, 0:1], in_=prod_t[:],
                    axis=mybir.AxisListType.X, op=mybir.AluOpType.add)
                nc.vector.tensor_reduce(
                    out=tsacc[:, g * GB:(g + 1) * GB, 1:2], in_=in_t[:],
                    axis=mybir.AxisListType.X, op=mybir.AluOpType.add)
            nc.sync.dma_start(out=tsloc_v, in_=tsacc[:])
    nc.compile()
    return nc


def build_p2():
    nc = bacc.Bacc()
    cand_d = nc.declare_dram_parameter("cand", [NO_PAD, KC, DIM], f32, isOutput=False)
    pk_d = nc.declare_dram_parameter("pk", [NO_PAD, KC, 4], f32, isOutput=False)
    v1r_d = nc.declare_dram_parameter("v1r", [P, DIM], f32, isOutput=False)
    oc_d = nc.declare_dram_parameter("oc", [NO_PAD, K, DIM], f32, isOutput=True)
    ondni_d = nc.declare_dram_parameter("ondni", [NO_PAD, K, 2], f32, isOutput=True)
    oab_d = nc.declare_dram_parameter("oab", [P, 2], f32, isOutput=True)

    NTILE = NO_PAD // P  # 20
    mult, add, sub = mybir.AluOpType.mult, mybir.AluOpType.add, mybir.AluOpType.subtract
    is_eq, is_lt = mybir.AluOpType.is_equal, mybir.AluOpType.is_lt

    with TileContext(nc) as tc:
        with (
            tc.tile_pool(name="const", bufs=1) as cpool,
            tc.tile_pool(name="cio", bufs=3) as cio,
            tc.tile_pool(name="oio", bufs=3) as oio,
            tc.tile_pool(name="work", bufs=2) as wp,
            tc.tile_pool(name="small", bufs=3) as sp,
        ):
            v1r_t = cpool.tile([P, DIM], f32)
            nc.sync.dma_start(out=v1r_t[:], in_=v1r_d[:])
            kio_i = cpool.tile([P, KC], i32)
            nc.gpsimd.iota(kio_i[:], pattern=[[1, KC]], base=0, channel_multiplier=0)
            kio_f = cpool.tile([P, KC], f32)
            nc.vector.tensor_copy(kio_f[:], kio_i[:])
            acc_a = cpool.tile([P, 1], f32)
            acc_b = cpool.tile([P, 1], f32)
            nc.vector.memset(acc_a[:], 0.0)
            nc.vector.memset(acc_b[:], 0.0)

            for t in range(NTILE):
                r0 = t * P
                cand_t = cio.tile([P, KC, DIM], f32, tag="cand")
                nc.sync.dma_start(out=cand_t[:], in_=cand_d[r0:r0 + P])
                pk_t = sp.tile([P, KC, 4], f32, tag="pk")
                nc.sync.dma_start(out=pk_t[:], in_=pk_d[r0:r0 + P])

                prod_t = wp.tile([P, KC, DIM], f32, tag="prod")
                nc.vector.tensor_tensor(
                    out=prod_t[:], in0=cand_t[:],
                    in1=v1r_t[:].unsqueeze(1).broadcast_to([P, KC, DIM]), op=mult)
                c_t = sp.tile([P, KC], f32, tag="c")
                nc.vector.tensor_reduce(out=c_t[:], in_=prod_t[:],
                                        axis=mybir.AxisListType.X, op=add)
                e_t = sp.tile([P, KC], f32, tag="e")
                nc.vector.tensor_tensor(
                    out=e_t[:], in0=c_t[:],
                    in1=pk_t[:, :, 0:1].rearrange("p k o -> p (k o)"), op=add)
                mn_t = sp.tile([P, 1], f32, tag="mn")
                nc.vector.tensor_reduce(out=mn_t[:], in_=e_t[:],
                                        axis=mybir.AxisListType.X,
                                        op=mybir.AluOpType.min)
                scr_t = sp.tile([P, KC], f32, tag="scr")
                m_t = sp.tile([P, 1], f32, tag="m")
                nc.vector.scalar_tensor_tensor(
                    out=scr_t[:], in0=e_t[:], scalar=mn_t[:], in1=kio_f[:],
                    op0=is_eq, op1=mult, accum_out=m_t[:])
                mask_t = sp.tile([P, K], f32, tag="mask")
                nc.vector.tensor_scalar(out=mask_t[:], in0=kio_f[:, 0:K],
                                        scalar1=m_t[:], scalar2=None, op0=is_lt)

                # candidate row selection: out[:,j,:] = B_j + mask_j*(A_j-B_j)
                diff_t = wp.tile([P, K, DIM], f32, tag="diff")
                nc.vector.tensor_tensor(out=diff_t[:], in0=cand_t[:, 0:K, :],
                                        in1=cand_t[:, 1:KC, :], op=sub)
                out_t = oio.tile([P, K, DIM], f32, tag="oc")
                for j in range(K):
                    nc.vector.scalar_tensor_tensor(
                        out=out_t[:, j, :], in0=diff_t[:, j, :],
                        scalar=mask_t[:, j:j + 1], in1=cand_t[:, j + 1, :],
                        op0=mult, op1=add)
                nc.sync.dma_start(out=oc_d[r0:r0 + P], in_=out_t[:])

                # nd / ni selection (both carried as f32 in pk cols 2,3)
                ndni_t = oio.tile([P, K, 2], f32, tag="ondni")
                dsc_t = sp.tile([P, K], f32, tag="dsc")
                msc_t = sp.tile([P, K], f32, tag="msc")
                for col, oidx in ((2, 0), (3, 1)):
                    A = pk_t[:, 0:K, col:col + 1].rearrange("p k o -> p (k o)")
                    B = pk_t[:, 1:KC, col:col + 1].rearrange("p k o -> p (k o)")
                    nc.vector.tensor_tensor(out=dsc_t[:], in0=A, in1=B, op=sub)
                    nc.vector.tensor_tensor(out=msc_t[:], in0=dsc_t[:],
                                            in1=mask_t[:], op=mult)
                    nc.vector.tensor_tensor(
                        out=ndni_t[:, :, oidx:oidx + 1].rearrange("p k o -> p (k o)"),
                        in0=msc_t[:], in1=B, op=add)
                nc.sync.dma_start(out=ondni_d[r0:r0 + P], in_=ndni_t[:])

                # a/b partial sums from s values (pk col 1)
                s_ap = pk_t[:, :, 1:2].rearrange("p k o -> p (k o)")
                bscr_t = sp.tile([P, KC], f32, tag="bscr")
                bp_t = sp.tile([P, 1], f32, tag="bp")
                nc.vector.scalar_tensor_tensor(
                    out=bscr_t[:], in0=kio_f[:], scalar=m_t[:], in1=s_ap,
                    op0=is_eq, op1=mult, accum_out=bp_t[:])
                ss_t = sp.tile([P, 1], f32, tag="ss")
                nc.vector.tensor_reduce(out=ss_t[:], in_=s_ap,
                                        axis=mybir.AxisListType.X, op=add)
                ap_t = sp.tile([P, 1], f32, tag="ap")
                nc.vector.tensor_tensor(out=ap_t[:], in0=ss_t[:], in1=bp_t[:], op=sub)
                nc.vector.tensor_tensor(out=acc_a[:], in0=acc_a[:], in1=ap_t[:], op=add)
                nc.vector.tensor_tensor(out=acc_b[:], in0=acc_b[:], in1=bp_t[:], op=add)

            ab_t = cpool.tile([P, 2], f32)
            nc.vector.tensor_copy(ab_t[:, 0:1], acc_a[:])
            nc.vector.tensor_copy(ab_t[:, 1:2], acc_b[:])
            nc.sync.dma_start(out=oab_d[:], in_=ab_t[:])
    nc.compile()
    return nc


def _run(nc, in_maps, tag):
    if TRACE["enabled"]:
        _ntff_hook()
        res = run_bass_kernel_spmd(nc, in_maps, core_ids=list(range(NCORES)),
                                   trace=True, tmpdir=f"/tmp/knn_trace_{tag}")
        TRACE["exec_ns"].append((tag, res.exec_time_ns))
        return res
    return run_bass_kernel_spmd(nc, in_maps, core_ids=list(range(NCORES)))


def kernel(X, Candidate, neigh_dist, neigh_ind, data_m_train, data_m_batch,
           test, W, A, **_unused):
    Candidate = np.ascontiguousarray(np.asarray(Candidate, dtype=np.float32))
    neigh_dist = np.ascontiguousarray(np.asarray(neigh_dist, dtype=np.float32))
    ni_in = np.asarray(neigh_ind)
    ni = ni_in.astype(np.int64)
    dmt = np.ascontiguousarray(np.asarray(data_m_train, dtype=np.float32))
    W = np.asarray(W, dtype=np.float32)
    A = np.asarray(A, dtype=np.float32)

    v = (W.astype(np.float64) @ A[OUT:, 0].astype(np.float64)).astype(np.float32)
    v1, v2 = v[:DIM], v[DIM:]
    v1r = np.ascontiguousarray(np.broadcast_to(v1, (P, DIM)))
    v2r = np.ascontiguousarray(np.broadcast_to(v2, (P, DIM)))

    # ---- program 1: (t, s) pair tables ----
    if "p1" not in _cache:
        _cache["p1"] = build_p1()
    dmt_pad = np.zeros((NCORES * NT_PAD, DIM), np.float32)
    for c in range(NCORES):
        dmt_pad[c * NT_PAD:c * NT_PAD + NT_SH] = dmt[c * NT_SH:(c + 1) * NT_SH]
    in1 = [{"dmt": dmt_pad[c * NT_PAD:(c + 1) * NT_PAD], "v2r": v2r}
           for c in range(NCORES)]
    res1 = _run(_cache["p1"], in1, "p1")

    # device wrote row (p*98+i) = shard row (i*128+p); invert that permutation
    r = np.arange(NT_SH)
    perm = (r % P) * NT_BLK + r // P
    ts_table = np.empty((NT, 2), np.float32)
    for c in range(NCORES):
        ts_table[c * NT_SH:(c + 1) * NT_SH] = res1.results[c]["tsloc"][perm]

    # ---- host glue: table lookup + packing ----
    tsg = ts_table[ni]                                   # [NO, KC, 2]
    pack = np.empty((NO_PAD * NCORES, KC, 4), np.float32)
    pack[:NO] = np.concatenate(
        [tsg, neigh_dist[:, :, None], ni.astype(np.float32)[:, :, None]], axis=2)
    # pad rows: zeros => zero contribution to a/b sums
    sh = np.zeros((NCORES, NO_PAD, KC, 4), np.float32)
    cand_sh = np.zeros((NCORES, NO_PAD, KC, DIM), np.float32)
    for c in range(NCORES):
        sh[c, :NO_SH] = pack[c * NO_SH:(c + 1) * NO_SH] if False else np.concatenate(
            [tsg[c * NO_SH:(c + 1) * NO_SH],
             neigh_dist[c * NO_SH:(c + 1) * NO_SH, :, None],
             ni[c * NO_SH:(c + 1) * NO_SH].astype(np.float32)[:, :, None]], axis=2)
        cand_sh[c, :NO_SH] = Candidate[c * NO_SH:(c + 1) * NO_SH]

    # ---- program 2 ----
    if "p2" not in _cache:
        _cache["p2"] = build_p2()
    in2 = [{"cand": cand_sh[c], "pk": sh[c], "v1r": v1r} for c in range(NCORES)]
    res2 = _run(_cache["p2"], in2, "p2")

    Cand_sel = np.empty((NO, K, DIM), np.float32)
    nd_out = np.empty((NO, K), np.float32)
    ni_f = np.empty((NO, K), np.float32)
    a_sum = 0.0
    b_sum = 0.0
    for c in range(NCORES):
        rr = res2.results[c]
        Cand_sel[c * NO_SH:(c + 1) * NO_SH] = rr["oc"][:NO_SH]
        nd_out[c * NO_SH:(c + 1) * NO_SH] = rr["ondni"][:NO_SH, :, 0]
        ni_f[c * NO_SH:(c + 1) * NO_SH] = rr["ondni"][:NO_SH, :, 1]
        a_sum += rr["oab"][:, 0].astype(np.float64).sum()
        b_sum += rr["oab"][:, 1].astype(np.float64).sum()

    ni_out = np.rint(ni_f).astype(ni_in.dtype)
    a_out = np.float32(a_sum / (NO * K))
    b_out = np.float32(b_sum / NO)
    return (Cand_sel, nd_out, ni_out, a_out, b_out)
